# revision 27
# baseline (speedup 1.0000x reference)
"""Trainium2 Bass kernel for nn_DLI_loss_full.

Key algebraic simplification: with logits(b,j,k) = a[b,j] + bp[b,k] + b_fc,
the per-pair loss  lse_j - pos_j  telescopes to

    log( sum_{k=j+1}^{L_b-1} exp(bp[b,k]) ) - bp[b,j+1]

i.e. the a[b,j] (LSTM path) and b_fc terms cancel exactly. The loss depends
only on bp[b,t] = segment_mean_t(encoder_output[b]) @ W_b, so the LSTM never
needs to run on device.

Summing over valid j (j <= L_b-2) per sample:

    loss_b = sum_j vmask_j * log(S_j) - sum_k 1{1<=k<=L_b-1} * bp_k
    S_j    = sum_k U[k,j] * (exp(bp_k) * 1{k<=L_b-1}),   U[k,j] = 1{k>j}

Device work per core (4 samples, data-parallel over batch):
  raw[t,:] = sum_s MT[s,t] * x[s,:]     (PE fp8 matmul; MT is an exact 0/1
                                         segment mask built on host, padded
                                         to 128 weight columns so the
                                         compiler's fast-weight-load kicks in)
  bp[t]    = inv_c[t] * raw[t,:]@W_b    (ACT drains PSUM, DVE mul+reduce with
                                         replicated W_b, then 1/count scale)
  epilogue = exp/mask/suffix-sum(matmul)/log/mask/reduce  (tiny)

Raggedness: rows past ends[b, L_b-1] contribute nothing, so sample b only
needs ceil((ends[b,L_b-1]+1)/128) row-chunks. Samples are sorted by need and
straight-dealt to cores (core c gets ranks c, 8+c, 16+c, 24+c), so program
slot k runs max-over-cores chunks for that rank octile -- near-optimal and
identical across cores (SPMD).

Stream layout: per <=8-chunk group, the padded MT block and the x block are
packed into ONE contiguous buffer so each group is a single ~1.2MiB DMA
(descriptor-gen on the sync queue is ~0.6us per dma_start, so few big DMAs).
x is quantized to fp8e4m3 on host: segment-mean averaging washes the
quantization noise out (measured loss rel err ~6e-6 vs fp32 reference).

Output: per-sample loss sums [4,1]; host sums across cores (order-invariant)
and divides by sum(L_b - 1).
"""

import os

import numpy as np
import ml_dtypes

import concourse.bass as bass
import concourse.bacc as bacc
import concourse.mybir as mybir
from concourse.tile import TileContext
from concourse.bass_utils import run_bass_kernel_spmd

N_CORES = 8
B, S, D, H, T = 32, 2048, 1024, 512, 64
BPC = B // N_CORES  # samples (slots) per core
NCHUNK = S // 128  # 16
GRP = 8  # chunks per DMA group
MTW = 64  # mt width per chunk
CW = MTW + D  # packed stream columns per chunk

_F32 = mybir.dt.float32
_X8 = mybir.dt.float8e4

# consts layout (free dim): umat, kmask, k1mask, vmask, pad, invc, ones
_C_UM = 0
_C_KM = T
_C_K1 = T + BPC
_C_VM = T + 2 * BPC
_C_PD = T + 3 * BPC
_C_IC = T + 4 * BPC
_C_ON = T + 5 * BPC
_C_W = T + 5 * BPC + 1

# set by test harness to enable HW profiling
last_exec_time_ns = None
_nc_cache = {}


def _groups(slot_chunks):
    """Yield (slot, group_chunk_offset_within_slot, glen, stream_col_offset)."""
    col = 0
    for b, nch in enumerate(slot_chunks):
        for g0 in range(0, nch, GRP):
            glen = min(GRP, nch - g0)
            yield b, g0, glen, col
            col += glen * CW


def _build_nc(slot_chunks):
    """slot_chunks: tuple of BPC ints -- chunks to process for each sample slot."""
    totc = sum(slot_chunks)
    nc = bacc.Bacc()
    # packed stream: per group [mt-pad block glen*MTW | x block glen*D]
    xm = nc.dram_tensor("xm", [128, totc * CW], _X8, kind="ExternalInput")
    # W_b replicated over turns: [T, D] f32
    wbr = nc.dram_tensor("wbr", [T, D], _F32, kind="ExternalInput")
    consts = nc.dram_tensor("consts", [T, _C_W], _F32, kind="ExternalInput")
    out = nc.dram_tensor("out", [BPC, 1], _F32, kind="ExternalOutput")

    with TileContext(nc) as tc:
        with (
            tc.tile_pool(name="xp", bufs=5) as xp,
            tc.tile_pool(name="cst", bufs=1) as cst,
            tc.tile_pool(name="sml", bufs=2) as sml,
            tc.tile_pool(name="ps", bufs=3, space="PSUM") as ps,
            tc.tile_pool(name="ps2", bufs=1, space="PSUM") as ps2,
        ):
            # small inputs on the scalar HWDGE queue; the stream owns sync
            cst_t = cst.tile([T, _C_W], _F32)
            nc.scalar.dma_start(out=cst_t[:], in_=consts[:])
            wbr_t = cst.tile([T, D], _F32)
            nc.scalar.dma_start(out=wbr_t[:], in_=wbr[:])

            # hoist Exp/Ln act-table loads off the epilogue critical path;
            # memset input so this doesn't wait on any DMA
            warm = sml.tile([T, 1], _F32, tag="warm")
            nc.gpsimd.memset(warm[:], 1.0)
            nc.scalar.activation(out=warm[:], in_=warm[:],
                                 func=mybir.ActivationFunctionType.Exp)
            nc.scalar.activation(out=warm[:], in_=warm[:],
                                 func=mybir.ActivationFunctionType.Ln)

            # warm the PE HAM clock gate during the initial DMA wait so real
            # matmuls run at 2.4GHz from the start (alternating col-tiles to
            # match the main loop's 128x64 array mode)
            wl = sml.tile([128, MTW], _X8, tag="wl")
            nc.gpsimd.memset(wl[:], 0.0)
            wr = sml.tile([128, 512], _X8, tag="wr")
            nc.gpsimd.memset(wr[:], 0.0)
            wps = ps2.tile([128, 512], _F32, tag="s_ps")  # shares the s_ps bank
            for wi in range(18):
                half = wi % 2
                nc.tensor.matmul(wps[64 * half : 64 * half + 64, :], lhsT=wl[:],
                                 rhs=wr[:], start=True, stop=True,
                                 tile_position=(0, 64 * half))

            bp_raw = cst.tile([T, BPC], _F32)
            slot_ps = {}
            for gi, (b, g0, glen, col) in enumerate(_groups(slot_chunks)):
                nch = slot_chunks[b]
                if g0 == 0:
                    slot_ps[b] = (
                        ps.tile([128, 512], _F32, tag="ps_a", name=f"ps_a{b}"),
                        ps.tile([128, 512], _F32, tag="ps_b", name=f"ps_b{b}"),
                    )
                ps_a, ps_b = slot_ps[b]
                gt = xp.tile([128, GRP * CW], _X8, tag="gt")
                nc.sync.dma_start(
                    out=gt[:, : glen * CW], in_=xm[:, col : col + glen * CW]
                )
                n_ev = (nch + 1) // 2
                n_od = nch // 2
                xoff = glen * MTW
                for cc in range(glen):
                    c = g0 + cc
                    lhs = gt[:, cc * MTW : (cc + 1) * MTW]
                    xcol = xoff + cc * D
                    half = c % 2  # even chunks -> array cols 0-63, odd -> 64-127
                    po = 64 * half
                    first = c < 2
                    last = c >= nch - 2
                    nc.tensor.matmul(
                        ps_a[po : po + 64, :], lhsT=lhs, rhs=gt[:, xcol : xcol + 512],
                        start=first, stop=last, tile_position=(0, po),
                    )
                    nc.tensor.matmul(
                        ps_b[po : po + 64, :], lhsT=lhs, rhs=gt[:, xcol + 512 : xcol + D],
                        start=first, stop=last, tile_position=(0, po),
                    )
                if g0 + glen == nch:
                    # slot done: sum the even/odd array-tile halves, then the
                    # W_b dot (DVE mul+reduce per bank)
                    prod = sml.tile([T, D], _F32, tag="prod")
                    acc_a = sml.tile([T, 1], _F32, tag="acc_a")
                    acc_b = sml.tile([T, 1], _F32, tag="acc_b")
                    if n_od:
                        # DVE can read only one PSUM operand; ACT (idle)
                        # drains the odd-tile half to SBUF first
                        cpa = sml.tile([T, 512], _F32, tag="cpa")
                        cpb = sml.tile([T, 512], _F32, tag="cpb")
                        nc.scalar.copy(out=cpa[:], in_=ps_a[64 : 64 + T, :])
                        nc.scalar.copy(out=cpb[:], in_=ps_b[64 : 64 + T, :])
                        sum_a = sml.tile([T, 512], _F32, tag="sum_a")
                        sum_b = sml.tile([T, 512], _F32, tag="sum_b")
                        nc.vector.tensor_add(out=sum_a[:], in0=ps_a[0:T, :], in1=cpa[:])
                        nc.vector.tensor_add(out=sum_b[:], in0=ps_b[0:T, :], in1=cpb[:])
                        in_a, in_b = sum_a, sum_b
                    else:
                        in_a, in_b = ps_a[0:T, :], ps_b[0:T, :]
                    nc.vector.tensor_mul(out=prod[:, 0:512], in0=in_a[:], in1=wbr_t[:, 0:512])
                    nc.vector.reduce_sum(out=acc_a[:], in_=prod[:, 0:512],
                                         axis=mybir.AxisListType.X)
                    nc.vector.tensor_mul(out=prod[:, 512:1024], in0=in_b[:],
                                         in1=wbr_t[:, 512:1024])
                    nc.vector.reduce_sum(out=acc_b[:], in_=prod[:, 512:1024],
                                         axis=mybir.AxisListType.X)
                    nc.vector.tensor_add(out=bp_raw[:, b : b + 1],
                                         in0=acc_a[:], in1=acc_b[:])

            # epilogue over all BPC samples at once: [T, BPC] tiles
            bp = sml.tile([T, BPC], _F32, tag="bp")
            nc.vector.tensor_mul(out=bp[:], in0=bp_raw[:], in1=cst_t[:, _C_IC : _C_IC + BPC])
            expd = sml.tile([T, BPC], _F32, tag="expd")
            nc.scalar.activation(out=expd[:], in_=bp[:], func=mybir.ActivationFunctionType.Exp)
            emask = sml.tile([T, BPC], _F32, tag="emask")
            nc.vector.tensor_mul(out=emask[:], in0=expd[:], in1=cst_t[:, _C_KM : _C_KM + BPC])
            s_ps = ps2.tile([T, BPC], _F32)
            nc.tensor.matmul(s_ps[:], lhsT=cst_t[:, _C_UM : _C_UM + T], rhs=emask[:],
                             start=True, stop=True)
            s_sb = sml.tile([T, BPC], _F32, tag="s_sb")
            nc.vector.tensor_add(out=s_sb[:], in0=s_ps[:], in1=cst_t[:, _C_PD : _C_PD + BPC])
            logs = sml.tile([T, BPC], _F32, tag="logs")
            nc.scalar.activation(out=logs[:], in_=s_sb[:], func=mybir.ActivationFunctionType.Ln)
            t1 = sml.tile([T, BPC], _F32, tag="t1")
            nc.vector.tensor_mul(out=t1[:], in0=logs[:], in1=cst_t[:, _C_VM : _C_VM + BPC])
            t2 = sml.tile([T, BPC], _F32, tag="t2")
            nc.vector.tensor_mul(out=t2[:], in0=bp[:], in1=cst_t[:, _C_K1 : _C_K1 + BPC])
            diff = sml.tile([T, BPC], _F32, tag="diff")
            nc.vector.tensor_sub(out=diff[:], in0=t1[:], in1=t2[:])
            o_ps = ps2.tile([BPC, 1], _F32)
            nc.tensor.matmul(o_ps[:], lhsT=diff[:], rhs=cst_t[:, _C_ON : _C_ON + 1],
                             start=True, stop=True)
            o_sb = sml.tile([BPC, 1], _F32, tag="o_sb")
            nc.scalar.copy(out=o_sb[:], in_=o_ps[:])
            nc.scalar.dma_start(out=out[:], in_=o_sb[:])

    nc.compile()
    return nc


def _host_prep(inputs):
    enc = np.asarray(inputs["encoder_output"], dtype=np.float32)
    ends = np.asarray(inputs["his_turn_end_ids"]).astype(np.int64)
    lens = np.asarray(inputs["turn_lengths"]).astype(np.int64)
    w_fc = np.asarray(inputs["W_fc"], dtype=np.float32)
    w_b = w_fc[0, H:]  # [D]

    # per-sample needed chunks; sort desc, straight-deal to cores
    need = np.array(
        [int(np.ceil((ends[b, lens[b] - 1] + 1) / 128)) for b in range(B)], np.int64
    )
    order = np.argsort(-need, kind="stable")  # rank -> sample
    assign = order.reshape(BPC, N_CORES)  # [slot, core]
    slot_chunks = tuple(int(need[assign[k]].max()) for k in range(BPC))
    totc = sum(slot_chunks)

    # fp8 cast + chunk swizzle: [B, 128, NCHUNK, D]
    enc_q = enc.astype(ml_dtypes.float8_e4m3)
    x_sw = enc_q.reshape(B, NCHUNK, 128, D).transpose(0, 2, 1, 3)  # [B,128,NCHUNK,D]

    starts = np.concatenate([np.zeros((B, 1), np.int64), ends[:, :-1] + 1], axis=1)
    counts = (ends - starts + 1).astype(np.float32)
    s_idx = np.arange(S, dtype=np.int64)[None, :, None]
    mt_full = (
        (s_idx >= starts[:, None, :])
        & (s_idx <= ends[:, None, :])
        & (np.arange(T)[None, None, :] < lens[:, None, None])
    ).astype(ml_dtypes.float8_e4m3)  # exact 0/1 in fp8
    mt_sw = mt_full.reshape(B, NCHUNK, 128, T).transpose(0, 2, 3, 1)  # [B,128,T,NCHUNK]

    wbr = np.ascontiguousarray(np.broadcast_to(w_b[None, :], (T, D)))
    umat = (np.arange(T)[:, None] > np.arange(T)[None, :]).astype(np.float32)
    t_idx = np.arange(T)[:, None]  # [T, 1]
    inv_c = (1.0 / counts) * (t_idx.T < lens[:, None])  # [B, T]

    in_maps = []
    for ci in range(N_CORES):
        samples = [int(assign[k, ci]) for k in range(BPC)]
        xs = np.zeros((128, totc * CW), ml_dtypes.float8_e4m3)
        for b, g0, glen, col in _groups(slot_chunks):
            sb = samples[b]
            mt_blk = xs[:, col : col + glen * MTW].reshape(128, glen, MTW)
            x_blk = xs[:, col + glen * MTW : col + glen * CW].reshape(128, glen, D)
            for cc in range(glen):
                c = g0 + cc
                mt_blk[:, cc, :] = mt_sw[sb, :, :, c]
                x_blk[:, cc, :] = x_sw[sb, :, c, :]
        lc = lens[samples][None, :]  # [1, BPC]
        consts = np.empty((T, _C_W), np.float32)
        consts[:, _C_UM : _C_UM + T] = umat
        consts[:, _C_KM : _C_KM + BPC] = t_idx <= lc - 1
        consts[:, _C_K1 : _C_K1 + BPC] = (t_idx >= 1) & (t_idx <= lc - 1)
        consts[:, _C_VM : _C_VM + BPC] = t_idx <= lc - 2
        consts[:, _C_PD : _C_PD + BPC] = t_idx >= lc - 1
        consts[:, _C_IC : _C_IC + BPC] = inv_c[samples].T
        consts[:, _C_ON] = 1.0
        in_maps.append({"xm": xs, "wbr": wbr, "consts": consts})
    return in_maps, lens, slot_chunks


def kernel(**inputs) -> np.ndarray:
    global last_exec_time_ns, _nc_cache

    in_maps, lens, slot_chunks = _host_prep(inputs)

    if slot_chunks not in _nc_cache:
        _nc_cache[slot_chunks] = _build_nc(slot_chunks)
    nc = _nc_cache[slot_chunks]

    trace = bool(int(os.environ.get("KERNEL_TRACE", "0")))
    res = None
    last_err = None
    for _attempt in range(4):
        t = trace and _attempt == 0  # profiler can't restart after a fault
        try:
            res = run_bass_kernel_spmd(
                nc,
                in_maps,
                list(range(N_CORES)),
                trace=t,
                trace_cores=list(range(N_CORES)) if t else None,
            )
            break
        except Exception as e:  # transient first-run NRT faults; retry
            last_err = e
    if res is None:
        raise last_err
    last_exec_time_ns = res.exec_time_ns

    total = np.float64(0.0)
    for ci in range(N_CORES):
        total += np.sum(res.results[ci]["out"].astype(np.float64))
    denom = float(np.sum(lens - 1))
    return np.asarray(np.float32(total / denom))


# revision 28
# speedup vs baseline: 1.0283x; 1.0283x over previous
"""Trainium2 Bass kernel for nn_DLI_loss_full.

Key algebraic simplification: with logits(b,j,k) = a[b,j] + bp[b,k] + b_fc,
the per-pair loss  lse_j - pos_j  telescopes to

    log( sum_{k=j+1}^{L_b-1} exp(bp[b,k]) ) - bp[b,j+1]

i.e. the a[b,j] (LSTM path) and b_fc terms cancel exactly. The loss depends
only on bp[b,t] = segment_mean_t(encoder_output[b]) @ W_b, so the LSTM never
needs to run on device.

Summing over valid j (j <= L_b-2) per sample:

    loss_b = sum_j vmask_j * log(S_j) - sum_k 1{1<=k<=L_b-1} * bp_k
    S_j    = sum_k U[k,j] * (exp(bp_k) * 1{k<=L_b-1}),   U[k,j] = 1{k>j}

Device work per core (4 samples, data-parallel over batch):
  raw[t,:] = sum_s MT[s,t] * x[s,:]     (PE fp8 matmul; MT is an exact 0/1
                                         segment mask built on host. Even/odd
                                         chunks run on PE column-tiles (0,0)/
                                         (0,64) so weight loads overlap the
                                         other tile's matmuls; the two PSUM
                                         row-halves are summed on DVE)
  bp[t]    = inv_c[t] * raw[t,:]@W_b    (ACT drains PSUM, DVE mul+reduce with
                                         replicated W_b, then 1/count scale)
  epilogue = exp/mask/suffix-sum(matmul)/log/mask/reduce  (tiny)

Raggedness: rows past ends[b, L_b-1] contribute nothing, so sample b only
needs ceil((ends[b,L_b-1]+1)/128) row-chunks. Samples are sorted by need and
straight-dealt to cores (core c gets ranks c, 8+c, 16+c, 24+c), so program
slot k runs max-over-cores chunks for that rank octile -- near-optimal and
identical across cores (SPMD).

Stream layout: per <=8-chunk group, the padded MT block and the x block are
packed into ONE contiguous buffer so each group is a single ~1.2MiB DMA
(descriptor-gen on the sync queue is ~0.6us per dma_start, so few big DMAs).
x is quantized to fp8e4m3 on host: segment-mean averaging washes the
quantization noise out (measured loss rel err ~6e-6 vs fp32 reference).

Output: per-sample loss sums [4,1]; host sums across cores (order-invariant)
and divides by sum(L_b - 1).
"""

import os

import numpy as np
import ml_dtypes

import concourse.bass as bass
import concourse.bacc as bacc
import concourse.mybir as mybir
from concourse.tile import TileContext
from concourse.bass_utils import run_bass_kernel_spmd

N_CORES = 8
B, S, D, H, T = 32, 2048, 1024, 512, 64
BPC = B // N_CORES  # samples (slots) per core
NCHUNK = S // 128  # 16
GRP = 8  # chunks per DMA group
MTW = 64  # mt width per chunk
CW = MTW + D  # packed stream columns per chunk

_F32 = mybir.dt.float32
_X8 = mybir.dt.float8e4

# consts layout (free dim): umat, kmask, k1mask, vmask, pad, invc, ones
_C_UM = 0
_C_KM = T
_C_K1 = T + BPC
_C_VM = T + 2 * BPC
_C_PD = T + 3 * BPC
_C_IC = T + 4 * BPC
_C_ON = T + 5 * BPC
_C_W = T + 5 * BPC + 1

# set by test harness to enable HW profiling
last_exec_time_ns = None
_nc_cache = {}


def _groups(slot_chunks):
    """Yield (slot, group_chunk_offset_within_slot, glen, stream_col_offset)."""
    col = 0
    for b, nch in enumerate(slot_chunks):
        for g0 in range(0, nch, GRP):
            glen = min(GRP, nch - g0)
            yield b, g0, glen, col
            col += glen * CW


def _build_nc(slot_chunks):
    """slot_chunks: tuple of BPC ints -- chunks to process for each sample slot."""
    totc = sum(slot_chunks)
    nc = bacc.Bacc()
    # packed stream: per group [mt-pad block glen*MTW | x block glen*D]
    xm = nc.dram_tensor("xm", [128, totc * CW], _X8, kind="ExternalInput")
    # W_b replicated over turns: [T, D] f32
    wbr = nc.dram_tensor("wbr", [T, D], _F32, kind="ExternalInput")
    consts = nc.dram_tensor("consts", [T, _C_W], _F32, kind="ExternalInput")
    out = nc.dram_tensor("out", [BPC, 1], _F32, kind="ExternalOutput")

    with TileContext(nc) as tc:
        with (
            tc.tile_pool(name="xp", bufs=5) as xp,
            tc.tile_pool(name="cst", bufs=1) as cst,
            tc.tile_pool(name="sml", bufs=2) as sml,
            tc.tile_pool(name="ps", bufs=3, space="PSUM") as ps,
            tc.tile_pool(name="ps2", bufs=1, space="PSUM") as ps2,
        ):
            # small inputs on the scalar HWDGE queue; the stream owns sync
            cst_t = cst.tile([T, _C_W], _F32)
            nc.scalar.dma_start(out=cst_t[:], in_=consts[:])
            wbr_t = cst.tile([T, D], _F32)
            nc.scalar.dma_start(out=wbr_t[:], in_=wbr[:])

            # hoist Exp/Ln act-table loads off the epilogue critical path;
            # memset input so this doesn't wait on any DMA
            warm = sml.tile([T, 1], _F32, tag="warm")
            nc.gpsimd.memset(warm[:], 1.0)
            nc.scalar.activation(out=warm[:], in_=warm[:],
                                 func=mybir.ActivationFunctionType.Exp)
            nc.scalar.activation(out=warm[:], in_=warm[:],
                                 func=mybir.ActivationFunctionType.Ln)

            # warm the PE HAM clock gate during the initial DMA wait so real
            # matmuls run at 2.4GHz from the start (alternating col-tiles to
            # match the main loop's 128x64 array mode)
            wl = sml.tile([128, MTW], _X8, tag="wl")
            nc.gpsimd.memset(wl[:], 0.0)
            wr = sml.tile([128, 512], _X8, tag="wr")
            nc.gpsimd.memset(wr[:], 0.0)
            wps = ps2.tile([128, 512], _F32, tag="s_ps")  # shares the s_ps bank
            for wi in range(18):
                half = wi % 2
                nc.tensor.matmul(wps[64 * half : 64 * half + 64, :], lhsT=wl[:],
                                 rhs=wr[:], start=True, stop=True,
                                 tile_position=(0, 64 * half))

            bp_raw = cst.tile([T, BPC], _F32)
            slot_ps = {}
            for gi, (b, g0, glen, col) in enumerate(_groups(slot_chunks)):
                nch = slot_chunks[b]
                if g0 == 0:
                    slot_ps[b] = (
                        ps.tile([128, 512], _F32, tag="ps_a", name=f"ps_a{b}"),
                        ps.tile([128, 512], _F32, tag="ps_b", name=f"ps_b{b}"),
                    )
                ps_a, ps_b = slot_ps[b]
                gt = xp.tile([128, GRP * CW], _X8, tag="gt")
                nc.sync.dma_start(
                    out=gt[:, : glen * CW], in_=xm[:, col : col + glen * CW]
                )
                n_ev = (nch + 1) // 2
                n_od = nch // 2
                xoff = glen * MTW
                for cc in range(glen):
                    c = g0 + cc
                    lhs = gt[:, cc * MTW : (cc + 1) * MTW]
                    xcol = xoff + cc * D
                    half = c % 2  # even chunks -> array cols 0-63, odd -> 64-127
                    po = 64 * half
                    first = c < 2
                    last = c >= nch - 2
                    nc.tensor.matmul(
                        ps_a[po : po + 64, :], lhsT=lhs, rhs=gt[:, xcol : xcol + 512],
                        start=first, stop=last, tile_position=(0, po),
                    )
                    nc.tensor.matmul(
                        ps_b[po : po + 64, :], lhsT=lhs, rhs=gt[:, xcol + 512 : xcol + D],
                        start=first, stop=last, tile_position=(0, po),
                    )
                if g0 + glen == nch:
                    # slot done: sum the even/odd array-tile halves, then the
                    # W_b dot (DVE mul+reduce per bank)
                    prod = sml.tile([T, D], _F32, tag="prod")
                    acc_a = sml.tile([T, 1], _F32, tag="acc_a")
                    acc_b = sml.tile([T, 1], _F32, tag="acc_b")
                    if n_od:
                        # DVE can read only one PSUM operand; ACT (idle)
                        # drains the odd-tile half to SBUF first
                        cpa = sml.tile([T, 512], _F32, tag="cpa")
                        cpb = sml.tile([T, 512], _F32, tag="cpb")
                        nc.scalar.copy(out=cpa[:], in_=ps_a[64 : 64 + T, :])
                        nc.scalar.copy(out=cpb[:], in_=ps_b[64 : 64 + T, :])
                        sum_a = sml.tile([T, 512], _F32, tag="sum_a")
                        sum_b = sml.tile([T, 512], _F32, tag="sum_b")
                        nc.vector.tensor_add(out=sum_a[:], in0=ps_a[0:T, :], in1=cpa[:])
                        nc.vector.tensor_add(out=sum_b[:], in0=ps_b[0:T, :], in1=cpb[:])
                        in_a, in_b = sum_a, sum_b
                    else:
                        in_a, in_b = ps_a[0:T, :], ps_b[0:T, :]
                    nc.vector.tensor_mul(out=prod[:, 0:512], in0=in_a[:], in1=wbr_t[:, 0:512])
                    nc.vector.reduce_sum(out=acc_a[:], in_=prod[:, 0:512],
                                         axis=mybir.AxisListType.X)
                    nc.vector.tensor_mul(out=prod[:, 512:1024], in0=in_b[:],
                                         in1=wbr_t[:, 512:1024])
                    nc.vector.reduce_sum(out=acc_b[:], in_=prod[:, 512:1024],
                                         axis=mybir.AxisListType.X)
                    nc.vector.tensor_add(out=bp_raw[:, b : b + 1],
                                         in0=acc_a[:], in1=acc_b[:])

            # epilogue over all BPC samples at once: [T, BPC] tiles
            bp = sml.tile([T, BPC], _F32, tag="bp")
            nc.vector.tensor_mul(out=bp[:], in0=bp_raw[:], in1=cst_t[:, _C_IC : _C_IC + BPC])
            expd = sml.tile([T, BPC], _F32, tag="expd")
            nc.scalar.activation(out=expd[:], in_=bp[:], func=mybir.ActivationFunctionType.Exp)
            emask = sml.tile([T, BPC], _F32, tag="emask")
            nc.vector.tensor_mul(out=emask[:], in0=expd[:], in1=cst_t[:, _C_KM : _C_KM + BPC])
            s_ps = ps2.tile([T, BPC], _F32)
            nc.tensor.matmul(s_ps[:], lhsT=cst_t[:, _C_UM : _C_UM + T], rhs=emask[:],
                             start=True, stop=True)
            s_sb = sml.tile([T, BPC], _F32, tag="s_sb")
            nc.vector.tensor_add(out=s_sb[:], in0=s_ps[:], in1=cst_t[:, _C_PD : _C_PD + BPC])
            logs = sml.tile([T, BPC], _F32, tag="logs")
            nc.scalar.activation(out=logs[:], in_=s_sb[:], func=mybir.ActivationFunctionType.Ln)
            t1 = sml.tile([T, BPC], _F32, tag="t1")
            nc.vector.tensor_mul(out=t1[:], in0=logs[:], in1=cst_t[:, _C_VM : _C_VM + BPC])
            t2 = sml.tile([T, BPC], _F32, tag="t2")
            nc.vector.tensor_mul(out=t2[:], in0=bp[:], in1=cst_t[:, _C_K1 : _C_K1 + BPC])
            diff = sml.tile([T, BPC], _F32, tag="diff")
            nc.vector.tensor_sub(out=diff[:], in0=t1[:], in1=t2[:])
            o_ps = ps2.tile([BPC, 1], _F32)
            nc.tensor.matmul(o_ps[:], lhsT=diff[:], rhs=cst_t[:, _C_ON : _C_ON + 1],
                             start=True, stop=True)
            o_sb = sml.tile([BPC, 1], _F32, tag="o_sb")
            nc.scalar.copy(out=o_sb[:], in_=o_ps[:])
            nc.scalar.dma_start(out=out[:], in_=o_sb[:])

    nc.compile()
    return nc


def _host_prep(inputs):
    enc = np.asarray(inputs["encoder_output"], dtype=np.float32)
    ends = np.asarray(inputs["his_turn_end_ids"]).astype(np.int64)
    lens = np.asarray(inputs["turn_lengths"]).astype(np.int64)
    w_fc = np.asarray(inputs["W_fc"], dtype=np.float32)
    w_b = w_fc[0, H:]  # [D]

    # per-sample needed chunks; sort desc, straight-deal to cores
    need = np.array(
        [int(np.ceil((ends[b, lens[b] - 1] + 1) / 128)) for b in range(B)], np.int64
    )
    order = np.argsort(-need, kind="stable")  # rank -> sample
    assign = order.reshape(BPC, N_CORES)  # [slot, core]
    slot_chunks = tuple(int(need[assign[k]].max()) for k in range(BPC))
    totc = sum(slot_chunks)

    # fp8 cast + chunk swizzle: [B, 128, NCHUNK, D]
    enc_q = enc.astype(ml_dtypes.float8_e4m3)
    x_sw = enc_q.reshape(B, NCHUNK, 128, D).transpose(0, 2, 1, 3)  # [B,128,NCHUNK,D]

    starts = np.concatenate([np.zeros((B, 1), np.int64), ends[:, :-1] + 1], axis=1)
    counts = (ends - starts + 1).astype(np.float32)
    s_idx = np.arange(S, dtype=np.int64)[None, :, None]
    mt_full = (
        (s_idx >= starts[:, None, :])
        & (s_idx <= ends[:, None, :])
        & (np.arange(T)[None, None, :] < lens[:, None, None])
    ).astype(ml_dtypes.float8_e4m3)  # exact 0/1 in fp8
    mt_sw = mt_full.reshape(B, NCHUNK, 128, T).transpose(0, 2, 3, 1)  # [B,128,T,NCHUNK]

    wbr = np.ascontiguousarray(np.broadcast_to(w_b[None, :], (T, D)))
    umat = (np.arange(T)[:, None] > np.arange(T)[None, :]).astype(np.float32)
    t_idx = np.arange(T)[:, None]  # [T, 1]
    inv_c = (1.0 / counts) * (t_idx.T < lens[:, None])  # [B, T]

    in_maps = []
    for ci in range(N_CORES):
        samples = [int(assign[k, ci]) for k in range(BPC)]
        xs = np.zeros((128, totc * CW), ml_dtypes.float8_e4m3)
        for b, g0, glen, col in _groups(slot_chunks):
            sb = samples[b]
            mt_blk = xs[:, col : col + glen * MTW].reshape(128, glen, MTW)
            x_blk = xs[:, col + glen * MTW : col + glen * CW].reshape(128, glen, D)
            for cc in range(glen):
                c = g0 + cc
                mt_blk[:, cc, :] = mt_sw[sb, :, :, c]
                x_blk[:, cc, :] = x_sw[sb, :, c, :]
        lc = lens[samples][None, :]  # [1, BPC]
        consts = np.empty((T, _C_W), np.float32)
        consts[:, _C_UM : _C_UM + T] = umat
        consts[:, _C_KM : _C_KM + BPC] = t_idx <= lc - 1
        consts[:, _C_K1 : _C_K1 + BPC] = (t_idx >= 1) & (t_idx <= lc - 1)
        consts[:, _C_VM : _C_VM + BPC] = t_idx <= lc - 2
        consts[:, _C_PD : _C_PD + BPC] = t_idx >= lc - 1
        consts[:, _C_IC : _C_IC + BPC] = inv_c[samples].T
        consts[:, _C_ON] = 1.0
        in_maps.append({"xm": xs, "wbr": wbr, "consts": consts})
    return in_maps, lens, slot_chunks


def kernel(**inputs) -> np.ndarray:
    global last_exec_time_ns, _nc_cache

    in_maps, lens, slot_chunks = _host_prep(inputs)

    if slot_chunks not in _nc_cache:
        _nc_cache[slot_chunks] = _build_nc(slot_chunks)
    nc = _nc_cache[slot_chunks]

    trace = bool(int(os.environ.get("KERNEL_TRACE", "0")))
    res = None
    last_err = None
    for _attempt in range(4):
        t = trace and _attempt == 0  # profiler can't restart after a fault
        try:
            res = run_bass_kernel_spmd(
                nc,
                in_maps,
                list(range(N_CORES)),
                trace=t,
                trace_cores=list(range(N_CORES)) if t else None,
            )
            break
        except Exception as e:  # transient first-run NRT faults; retry
            last_err = e
    if res is None:
        raise last_err
    last_exec_time_ns = res.exec_time_ns

    total = np.float64(0.0)
    for ci in range(N_CORES):
        total += np.sum(res.results[ci]["out"].astype(np.float64))
    denom = float(np.sum(lens - 1))
    return np.asarray(np.float32(total / denom))


# revision 29
# speedup vs baseline: 1.0434x; 1.0146x over previous
"""Trainium2 Bass kernel for nn_DLI_loss_full.

Key algebraic simplification: with logits(b,j,k) = a[b,j] + bp[b,k] + b_fc,
the per-pair loss  lse_j - pos_j  telescopes to

    log( sum_{k=j+1}^{L_b-1} exp(bp[b,k]) ) - bp[b,j+1]

i.e. the a[b,j] (LSTM path) and b_fc terms cancel exactly. The loss depends
only on bp[b,t] = segment_mean_t(encoder_output[b]) @ W_b, so the LSTM never
needs to run on device.

Summing over valid j (j <= L_b-2) per sample:

    loss_b = sum_j vmask_j * log(S_j) - sum_k 1{1<=k<=L_b-1} * bp_k
    S_j    = sum_k U[k,j] * (exp(bp_k) * 1{k<=L_b-1}),   U[k,j] = 1{k>j}

Device work per core (4 samples, data-parallel over batch):
  raw[t,:] = sum_s MT[s,t] * x[s,:]     (PE fp8 matmul; MT is an exact 0/1
                                         segment mask built on host. Even/odd
                                         chunks run on PE column-tiles (0,0)/
                                         (0,64) so weight loads overlap the
                                         other tile's matmuls; the two PSUM
                                         row-halves are summed on DVE)
  bp[t]    = inv_c[t] * raw[t,:]@W_b    (ACT drains PSUM, DVE mul+reduce with
                                         replicated W_b, then 1/count scale)
  epilogue = exp/mask/suffix-sum(matmul)/log/mask/reduce  (tiny)

Raggedness: rows past ends[b, L_b-1] contribute nothing, so sample b only
needs ceil((ends[b,L_b-1]+1)/128) row-chunks. Samples are sorted by need and
straight-dealt to cores (core c gets ranks c, 8+c, 16+c, 24+c), so program
slot k runs max-over-cores chunks for that rank octile -- near-optimal and
identical across cores (SPMD).

Stream layout: per <=8-chunk group, the padded MT block and the x block are
packed into ONE contiguous buffer so each group is a single ~1.2MiB DMA
(descriptor-gen on the sync queue is ~0.6us per dma_start, so few big DMAs).
x is quantized to fp8e4m3 on host: segment-mean averaging washes the
quantization noise out (measured loss rel err ~6e-6 vs fp32 reference).

Output: per-sample loss sums [4,1]; host sums across cores (order-invariant)
and divides by sum(L_b - 1).
"""

import os

import numpy as np
import ml_dtypes

import concourse.bass as bass
import concourse.bacc as bacc
import concourse.mybir as mybir
from concourse.tile import TileContext
from concourse.bass_utils import run_bass_kernel_spmd

N_CORES = 8
B, S, D, H, T = 32, 2048, 1024, 512, 64
BPC = B // N_CORES  # samples (slots) per core
NCHUNK = S // 128  # 16
GRP = 8  # chunks per DMA group
MTW = 64  # mt width per chunk
CW = MTW + D  # packed stream columns per chunk

_F32 = mybir.dt.float32
_X8 = mybir.dt.float8e4

# consts layout (free dim): umat, kmask, k1mask, vmask, pad, invc, ones
_C_UM = 0
_C_KM = T
_C_K1 = T + BPC
_C_VM = T + 2 * BPC
_C_PD = T + 3 * BPC
_C_IC = T + 4 * BPC
_C_ON = T + 5 * BPC
_C_W = T + 5 * BPC + 1

# set by test harness to enable HW profiling
last_exec_time_ns = None
_nc_cache = {}


def _groups(slot_chunks):
    """Yield (slot, group_chunk_offset_within_slot, glen, stream_col_offset)."""
    col = 0
    for b, nch in enumerate(slot_chunks):
        for g0 in range(0, nch, GRP):
            glen = min(GRP, nch - g0)
            yield b, g0, glen, col
            col += glen * CW


def _build_nc(slot_chunks):
    """slot_chunks: tuple of BPC ints -- chunks to process for each sample slot."""
    totc = sum(slot_chunks)
    nc = bacc.Bacc()
    # packed stream: per group [mt-pad block glen*MTW | x block glen*D]
    xm = nc.dram_tensor("xm", [128, totc * CW], _X8, kind="ExternalInput")
    # W_b replicated over all 128 partitions: [128, D] f32
    wbr = nc.dram_tensor("wbr", [128, D], _F32, kind="ExternalInput")
    consts = nc.dram_tensor("consts", [T, _C_W], _F32, kind="ExternalInput")
    out = nc.dram_tensor("out", [BPC, 1], _F32, kind="ExternalOutput")

    with TileContext(nc) as tc:
        with (
            tc.tile_pool(name="xp", bufs=5) as xp,
            tc.tile_pool(name="cst", bufs=1) as cst,
            tc.tile_pool(name="sml", bufs=2) as sml,
            tc.tile_pool(name="ps", bufs=3, space="PSUM") as ps,
            tc.tile_pool(name="ps2", bufs=1, space="PSUM") as ps2,
        ):
            # small inputs on the scalar HWDGE queue; the stream owns sync
            cst_t = cst.tile([T, _C_W], _F32)
            nc.scalar.dma_start(out=cst_t[:], in_=consts[:])
            wbr_t = cst.tile([128, D], _F32)
            nc.scalar.dma_start(out=wbr_t[:], in_=wbr[:])

            # hoist Exp/Ln act-table loads off the epilogue critical path;
            # memset input so this doesn't wait on any DMA
            warm = sml.tile([T, 1], _F32, tag="warm")
            nc.gpsimd.memset(warm[:], 1.0)
            nc.scalar.activation(out=warm[:], in_=warm[:],
                                 func=mybir.ActivationFunctionType.Exp)
            nc.scalar.activation(out=warm[:], in_=warm[:],
                                 func=mybir.ActivationFunctionType.Ln)

            # warm the PE HAM clock gate during the initial DMA wait so real
            # matmuls run at 2.4GHz from the start (alternating col-tiles to
            # match the main loop's 128x64 array mode)
            wl = sml.tile([128, MTW], _X8, tag="wl")
            nc.gpsimd.memset(wl[:], 0.0)
            wr = sml.tile([128, 512], _X8, tag="wr")
            nc.gpsimd.memset(wr[:], 0.0)
            wps = ps2.tile([128, 512], _F32, tag="s_ps")  # shares the s_ps bank
            for wi in range(18):
                half = wi % 2
                nc.tensor.matmul(wps[64 * half : 64 * half + 64, :], lhsT=wl[:],
                                 rhs=wr[:], start=True, stop=True,
                                 tile_position=(0, 64 * half))

            bp_raw = cst.tile([T, BPC], _F32)
            slot_ps = {}
            for gi, (b, g0, glen, col) in enumerate(_groups(slot_chunks)):
                nch = slot_chunks[b]
                if g0 == 0:
                    slot_ps[b] = (
                        ps.tile([128, 512], _F32, tag="ps_a", name=f"ps_a{b}"),
                        ps.tile([128, 512], _F32, tag="ps_b", name=f"ps_b{b}"),
                    )
                ps_a, ps_b = slot_ps[b]
                gt = xp.tile([128, GRP * CW], _X8, tag="gt")
                nc.sync.dma_start(
                    out=gt[:, : glen * CW], in_=xm[:, col : col + glen * CW]
                )
                n_ev = (nch + 1) // 2
                n_od = nch // 2
                xoff = glen * MTW
                for cc in range(glen):
                    c = g0 + cc
                    lhs = gt[:, cc * MTW : (cc + 1) * MTW]
                    xcol = xoff + cc * D
                    half = c % 2  # even chunks -> array cols 0-63, odd -> 64-127
                    po = 64 * half
                    first = c < 2
                    last = c >= nch - 2
                    nc.tensor.matmul(
                        ps_a[po : po + 64, :], lhsT=lhs, rhs=gt[:, xcol : xcol + 512],
                        start=first, stop=last, tile_position=(0, po),
                    )
                    nc.tensor.matmul(
                        ps_b[po : po + 64, :], lhsT=lhs, rhs=gt[:, xcol + 512 : xcol + D],
                        start=first, stop=last, tile_position=(0, po),
                    )
                if g0 + glen == nch:
                    # slot done: full-lane [128,512] mul+reduce per bank reads
                    # PSUM directly; rows 0:64 hold the even col-tile sums,
                    # 64:128 the odd ones, so tiny [64,1] adds finish the dot
                    np_ = 128 if n_od else T
                    prod = sml.tile([128, D], _F32, tag="prod")
                    acc_a = sml.tile([128, 1], _F32, tag="acc_a")
                    acc_b = sml.tile([128, 1], _F32, tag="acc_b")
                    nc.vector.tensor_mul(out=prod[0:np_, 0:512], in0=ps_a[0:np_, :],
                                         in1=wbr_t[0:np_, 0:512])
                    nc.vector.reduce_sum(out=acc_a[0:np_, :], in_=prod[0:np_, 0:512],
                                         axis=mybir.AxisListType.X)
                    nc.vector.tensor_mul(out=prod[0:np_, 512:1024], in0=ps_b[0:np_, :],
                                         in1=wbr_t[0:np_, 512:1024])
                    nc.vector.reduce_sum(out=acc_b[0:np_, :], in_=prod[0:np_, 512:1024],
                                         axis=mybir.AxisListType.X)
                    tsum = sml.tile([T, 1], _F32, tag="tsum")
                    nc.vector.tensor_add(out=tsum[:], in0=acc_a[0:T, :], in1=acc_b[0:T, :])
                    if n_od:
                        tsum2 = sml.tile([T, 1], _F32, tag="tsum2")
                        nc.vector.tensor_add(out=tsum2[:], in0=acc_a[64 : 64 + T, :],
                                             in1=acc_b[64 : 64 + T, :])
                        nc.vector.tensor_add(out=bp_raw[:, b : b + 1],
                                             in0=tsum[:], in1=tsum2[:])
                    else:
                        nc.vector.tensor_copy(out=bp_raw[:, b : b + 1], in_=tsum[:])

            # epilogue over all BPC samples at once: [T, BPC] tiles
            bp = sml.tile([T, BPC], _F32, tag="bp")
            nc.vector.tensor_mul(out=bp[:], in0=bp_raw[:], in1=cst_t[:, _C_IC : _C_IC + BPC])
            expd = sml.tile([T, BPC], _F32, tag="expd")
            nc.scalar.activation(out=expd[:], in_=bp[:], func=mybir.ActivationFunctionType.Exp)
            emask = sml.tile([T, BPC], _F32, tag="emask")
            nc.vector.tensor_mul(out=emask[:], in0=expd[:], in1=cst_t[:, _C_KM : _C_KM + BPC])
            s_ps = ps2.tile([T, BPC], _F32)
            nc.tensor.matmul(s_ps[:], lhsT=cst_t[:, _C_UM : _C_UM + T], rhs=emask[:],
                             start=True, stop=True)
            s_sb = sml.tile([T, BPC], _F32, tag="s_sb")
            nc.vector.tensor_add(out=s_sb[:], in0=s_ps[:], in1=cst_t[:, _C_PD : _C_PD + BPC])
            logs = sml.tile([T, BPC], _F32, tag="logs")
            nc.scalar.activation(out=logs[:], in_=s_sb[:], func=mybir.ActivationFunctionType.Ln)
            t1 = sml.tile([T, BPC], _F32, tag="t1")
            nc.vector.tensor_mul(out=t1[:], in0=logs[:], in1=cst_t[:, _C_VM : _C_VM + BPC])
            t2 = sml.tile([T, BPC], _F32, tag="t2")
            nc.vector.tensor_mul(out=t2[:], in0=bp[:], in1=cst_t[:, _C_K1 : _C_K1 + BPC])
            diff = sml.tile([T, BPC], _F32, tag="diff")
            nc.vector.tensor_sub(out=diff[:], in0=t1[:], in1=t2[:])
            o_ps = ps2.tile([BPC, 1], _F32)
            nc.tensor.matmul(o_ps[:], lhsT=diff[:], rhs=cst_t[:, _C_ON : _C_ON + 1],
                             start=True, stop=True)
            o_sb = sml.tile([BPC, 1], _F32, tag="o_sb")
            nc.scalar.copy(out=o_sb[:], in_=o_ps[:])
            nc.scalar.dma_start(out=out[:], in_=o_sb[:])

    nc.compile()
    return nc


def _host_prep(inputs):
    enc = np.asarray(inputs["encoder_output"], dtype=np.float32)
    ends = np.asarray(inputs["his_turn_end_ids"]).astype(np.int64)
    lens = np.asarray(inputs["turn_lengths"]).astype(np.int64)
    w_fc = np.asarray(inputs["W_fc"], dtype=np.float32)
    w_b = w_fc[0, H:]  # [D]

    # per-sample needed chunks; sort desc, straight-deal to cores
    need = np.array(
        [int(np.ceil((ends[b, lens[b] - 1] + 1) / 128)) for b in range(B)], np.int64
    )
    order = np.argsort(-need, kind="stable")  # rank -> sample
    assign = order.reshape(BPC, N_CORES)  # [slot, core]
    slot_chunks = tuple(int(need[assign[k]].max()) for k in range(BPC))
    totc = sum(slot_chunks)

    # fp8 cast + chunk swizzle: [B, 128, NCHUNK, D]
    enc_q = enc.astype(ml_dtypes.float8_e4m3)
    x_sw = enc_q.reshape(B, NCHUNK, 128, D).transpose(0, 2, 1, 3)  # [B,128,NCHUNK,D]

    starts = np.concatenate([np.zeros((B, 1), np.int64), ends[:, :-1] + 1], axis=1)
    counts = (ends - starts + 1).astype(np.float32)
    s_idx = np.arange(S, dtype=np.int64)[None, :, None]
    mt_full = (
        (s_idx >= starts[:, None, :])
        & (s_idx <= ends[:, None, :])
        & (np.arange(T)[None, None, :] < lens[:, None, None])
    ).astype(ml_dtypes.float8_e4m3)  # exact 0/1 in fp8
    mt_sw = mt_full.reshape(B, NCHUNK, 128, T).transpose(0, 2, 3, 1)  # [B,128,T,NCHUNK]

    wbr = np.ascontiguousarray(np.broadcast_to(w_b[None, :], (128, D)))
    umat = (np.arange(T)[:, None] > np.arange(T)[None, :]).astype(np.float32)
    t_idx = np.arange(T)[:, None]  # [T, 1]
    inv_c = (1.0 / counts) * (t_idx.T < lens[:, None])  # [B, T]

    in_maps = []
    for ci in range(N_CORES):
        samples = [int(assign[k, ci]) for k in range(BPC)]
        xs = np.zeros((128, totc * CW), ml_dtypes.float8_e4m3)
        for b, g0, glen, col in _groups(slot_chunks):
            sb = samples[b]
            mt_blk = xs[:, col : col + glen * MTW].reshape(128, glen, MTW)
            x_blk = xs[:, col + glen * MTW : col + glen * CW].reshape(128, glen, D)
            for cc in range(glen):
                c = g0 + cc
                mt_blk[:, cc, :] = mt_sw[sb, :, :, c]
                x_blk[:, cc, :] = x_sw[sb, :, c, :]
        lc = lens[samples][None, :]  # [1, BPC]
        consts = np.empty((T, _C_W), np.float32)
        consts[:, _C_UM : _C_UM + T] = umat
        consts[:, _C_KM : _C_KM + BPC] = t_idx <= lc - 1
        consts[:, _C_K1 : _C_K1 + BPC] = (t_idx >= 1) & (t_idx <= lc - 1)
        consts[:, _C_VM : _C_VM + BPC] = t_idx <= lc - 2
        consts[:, _C_PD : _C_PD + BPC] = t_idx >= lc - 1
        consts[:, _C_IC : _C_IC + BPC] = inv_c[samples].T
        consts[:, _C_ON] = 1.0
        in_maps.append({"xm": xs, "wbr": wbr, "consts": consts})
    return in_maps, lens, slot_chunks


def kernel(**inputs) -> np.ndarray:
    global last_exec_time_ns, _nc_cache

    in_maps, lens, slot_chunks = _host_prep(inputs)

    if slot_chunks not in _nc_cache:
        _nc_cache[slot_chunks] = _build_nc(slot_chunks)
    nc = _nc_cache[slot_chunks]

    trace = bool(int(os.environ.get("KERNEL_TRACE", "0")))
    res = None
    last_err = None
    for _attempt in range(4):
        t = trace and _attempt == 0  # profiler can't restart after a fault
        try:
            res = run_bass_kernel_spmd(
                nc,
                in_maps,
                list(range(N_CORES)),
                trace=t,
                trace_cores=list(range(N_CORES)) if t else None,
            )
            break
        except Exception as e:  # transient first-run NRT faults; retry
            last_err = e
    if res is None:
        raise last_err
    last_exec_time_ns = res.exec_time_ns

    total = np.float64(0.0)
    for ci in range(N_CORES):
        total += np.sum(res.results[ci]["out"].astype(np.float64))
    denom = float(np.sum(lens - 1))
    return np.asarray(np.float32(total / denom))


# revision 31
# speedup vs baseline: 1.1553x; 1.1073x over previous
"""Trainium2 Bass kernel for nn_DLI_loss_full.

Key algebraic simplification: with logits(b,j,k) = a[b,j] + bp[b,k] + b_fc,
the per-pair loss  lse_j - pos_j  telescopes to

    log( sum_{k=j+1}^{L_b-1} exp(bp[b,k]) ) - bp[b,j+1]

i.e. the a[b,j] (LSTM path) and b_fc terms cancel exactly. The loss depends
only on bp[b,t] = segment_mean_t(encoder_output[b]) @ W_b, so the LSTM never
needs to run on device.

Summing over valid j (j <= L_b-2) per sample:

    loss_b = sum_j vmask_j * log(S_j) - sum_k 1{1<=k<=L_b-1} * bp_k
    S_j    = sum_k U[k,j] * (exp(bp_k) * 1{k<=L_b-1}),   U[k,j] = 1{k>j}

Device work per core (4 samples, data-parallel over batch):
  raw[t,:] = sum_s MT[s,t] * x[s,:]     (PE fp8 matmul; MT is an exact 0/1
                                         segment mask built on host. Even/odd
                                         chunks run on PE column-tiles (0,0)/
                                         (0,64) so weight loads overlap the
                                         other tile's matmuls; the two PSUM
                                         row-halves are summed on DVE)
  bp[t]    = inv_c[t] * raw[t,:]@W_b    (full-lane [128,512] DVE mul+reduce
                                         straight from PSUM against W_b
                                         replicated on all partitions, tiny
                                         adds merge the col-tile halves,
                                         then 1/count scale)
  epilogue = exp/mask/suffix-sum(matmul)/log/mask/reduce  (tiny)

Raggedness: rows past ends[b, L_b-1] contribute nothing, so sample b only
needs ceil((ends[b,L_b-1]+1)/128) row-chunks. Samples are sorted by need and
straight-dealt to cores (core c gets ranks c, 8+c, 16+c, 24+c), so program
slot k runs max-over-cores chunks for that rank octile -- near-optimal and
identical across cores (SPMD).

Stream layout: per <=8-chunk group, the padded MT block and the x block are
packed into ONE contiguous buffer so each group is a single ~1.2MiB DMA
(descriptor-gen on the sync queue is ~0.6us per dma_start, so few big DMAs).
x is quantized to fp8e4m3 on host: segment-mean averaging washes the
quantization noise out (measured loss rel err ~6e-6 vs fp32 reference).

Output: per-sample loss sums [4,1]; host sums across cores (order-invariant)
and divides by sum(L_b - 1).
"""

import os

import numpy as np
import ml_dtypes

import concourse.bass as bass
import concourse.bacc as bacc
import concourse.mybir as mybir
from concourse.tile import TileContext
from concourse.bass_utils import run_bass_kernel_spmd

N_CORES = 8
B, S, D, H, T = 32, 2048, 1024, 512, 64
BPC = B // N_CORES  # samples (slots) per core
NCHUNK = S // 128  # 16
GRP = 8  # chunks per DMA group
MTW = 64  # mt width per chunk
CW = MTW + D  # packed stream columns per chunk

_F32 = mybir.dt.float32
_X8 = mybir.dt.float8e4

# consts layout (free dim): umat, kmask, k1mask, vmask, pad, invc, ones
_C_UM = 0
_C_KM = T
_C_K1 = T + BPC
_C_VM = T + 2 * BPC
_C_PD = T + 3 * BPC
_C_IC = T + 4 * BPC
_C_ON = T + 5 * BPC
_C_W = T + 5 * BPC + 1

# set by test harness to enable HW profiling
last_exec_time_ns = None
_nc_cache = {}


def _groups(slot_chunks):
    """Yield (slot, group_chunk_offset_within_slot, glen, stream_col_offset)."""
    col = 0
    for b, nch in enumerate(slot_chunks):
        for g0 in range(0, nch, GRP):
            glen = min(GRP, nch - g0)
            yield b, g0, glen, col
            col += glen * CW


def _build_nc(slot_chunks):
    """slot_chunks: tuple of BPC ints -- chunks to process for each sample slot."""
    totc = sum(slot_chunks)
    nc = bacc.Bacc()
    # packed stream: per group [mt-pad block glen*MTW | x block glen*D]
    xm = nc.dram_tensor("xm", [128, totc * CW], _X8, kind="ExternalInput")
    # W_b replicated over all 128 partitions: [128, D] f32
    wbr = nc.dram_tensor("wbr", [128, D], _F32, kind="ExternalInput")
    consts = nc.dram_tensor("consts", [T, _C_W], _F32, kind="ExternalInput")
    out = nc.dram_tensor("out", [BPC, 1], _F32, kind="ExternalOutput")

    with TileContext(nc) as tc:
        with (
            tc.tile_pool(name="xp", bufs=5) as xp,
            tc.tile_pool(name="cst", bufs=1) as cst,
            tc.tile_pool(name="sml", bufs=2) as sml,
            tc.tile_pool(name="ps", bufs=3, space="PSUM") as ps,
            tc.tile_pool(name="ps2", bufs=1, space="PSUM") as ps2,
        ):
            # small inputs on the scalar HWDGE queue; the stream owns sync
            cst_t = cst.tile([T, _C_W], _F32)
            nc.scalar.dma_start(out=cst_t[:], in_=consts[:])
            wbr_t = cst.tile([128, D], _F32)
            nc.scalar.dma_start(out=wbr_t[:], in_=wbr[:])

            # hoist Exp/Ln act-table loads off the epilogue critical path;
            # memset input so this doesn't wait on any DMA
            warm = sml.tile([T, 1], _F32, tag="warm")
            nc.gpsimd.memset(warm[:], 1.0)
            nc.scalar.activation(out=warm[:], in_=warm[:],
                                 func=mybir.ActivationFunctionType.Exp)
            nc.scalar.activation(out=warm[:], in_=warm[:],
                                 func=mybir.ActivationFunctionType.Ln)

            # warm the PE HAM clock gate during the initial DMA wait so real
            # matmuls run at 2.4GHz from the start (alternating col-tiles to
            # match the main loop's 128x64 array mode)
            wl = sml.tile([128, MTW], _X8, tag="wl")
            nc.gpsimd.memset(wl[:], 0.0)
            wr = sml.tile([128, 512], _X8, tag="wr")
            nc.gpsimd.memset(wr[:], 0.0)
            wps = ps2.tile([128, 512], _F32, tag="s_ps")  # shares the s_ps bank
            for wi in range(18):
                half = wi % 2
                nc.tensor.matmul(wps[64 * half : 64 * half + 64, :], lhsT=wl[:],
                                 rhs=wr[:], start=True, stop=True,
                                 tile_position=(0, 64 * half))

            bp_raw = cst.tile([T, BPC], _F32)
            slot_ps = {}
            for gi, (b, g0, glen, col) in enumerate(_groups(slot_chunks)):
                nch = slot_chunks[b]
                if g0 == 0:
                    slot_ps[b] = (
                        ps.tile([128, 512], _F32, tag="ps_a", name=f"ps_a{b}"),
                        ps.tile([128, 512], _F32, tag="ps_b", name=f"ps_b{b}"),
                    )
                ps_a, ps_b = slot_ps[b]
                gt = xp.tile([128, GRP * CW], _X8, tag="gt")
                nc.sync.dma_start(
                    out=gt[:, : glen * CW], in_=xm[:, col : col + glen * CW]
                )
                n_ev = (nch + 1) // 2
                n_od = nch // 2
                xoff = glen * MTW
                banks = [(ps_a, 0), (ps_b, 512)]
                if b == BPC - 1:
                    # last slot: finish bank A first so its dot overlaps
                    # bank B's matmuls (shortens the exposed tail)
                    for pst, doff in banks:
                        for cc in range(glen):
                            c = g0 + cc
                            po = 64 * (c % 2)
                            nc.tensor.matmul(
                                pst[po : po + 64, :],
                                lhsT=gt[:, cc * MTW : (cc + 1) * MTW],
                                rhs=gt[:, xoff + cc * D + doff : xoff + cc * D + doff + 512],
                                start=c < 2, stop=c >= nch - 2,
                                tile_position=(0, po),
                            )
                else:
                    for cc in range(glen):
                        c = g0 + cc
                        lhs = gt[:, cc * MTW : (cc + 1) * MTW]
                        xcol = xoff + cc * D
                        po = 64 * (c % 2)
                        first = c < 2
                        last = c >= nch - 2
                        nc.tensor.matmul(
                            ps_a[po : po + 64, :], lhsT=lhs, rhs=gt[:, xcol : xcol + 512],
                            start=first, stop=last, tile_position=(0, po),
                        )
                        nc.tensor.matmul(
                            ps_b[po : po + 64, :], lhsT=lhs, rhs=gt[:, xcol + 512 : xcol + D],
                            start=first, stop=last, tile_position=(0, po),
                        )
                if g0 + glen == nch:
                    # slot done: full-lane [128,512] mul+reduce per bank reads
                    # PSUM directly; rows 0:64 hold the even col-tile sums,
                    # 64:128 the odd ones, so tiny [64,1] adds finish the dot
                    np_ = 128 if n_od else T
                    prod = sml.tile([128, D], mybir.dt.bfloat16, tag="prod")
                    acc_a = sml.tile([128, 1], _F32, tag="acc_a")
                    acc_b = sml.tile([128, 1], _F32, tag="acc_b")
                    nc.vector.tensor_mul(out=prod[0:np_, 0:512], in0=ps_a[0:np_, :],
                                         in1=wbr_t[0:np_, 0:512])
                    nc.vector.reduce_sum(out=acc_a[0:np_, :], in_=prod[0:np_, 0:512],
                                         axis=mybir.AxisListType.X)
                    nc.vector.tensor_mul(out=prod[0:np_, 512:1024], in0=ps_b[0:np_, :],
                                         in1=wbr_t[0:np_, 512:1024])
                    nc.vector.reduce_sum(out=acc_b[0:np_, :], in_=prod[0:np_, 512:1024],
                                         axis=mybir.AxisListType.X)
                    tsum = sml.tile([T, 1], _F32, tag="tsum")
                    nc.vector.tensor_add(out=tsum[:], in0=acc_a[0:T, :], in1=acc_b[0:T, :])
                    if n_od:
                        tsum2 = sml.tile([T, 1], _F32, tag="tsum2")
                        nc.vector.tensor_add(out=tsum2[:], in0=acc_a[64 : 64 + T, :],
                                             in1=acc_b[64 : 64 + T, :])
                        nc.vector.tensor_add(out=bp_raw[:, b : b + 1],
                                             in0=tsum[:], in1=tsum2[:])
                    else:
                        nc.vector.tensor_copy(out=bp_raw[:, b : b + 1], in_=tsum[:])

            # epilogue over all BPC samples at once: [T, BPC] tiles
            bp = sml.tile([T, BPC], _F32, tag="bp")
            nc.vector.tensor_mul(out=bp[:], in0=bp_raw[:], in1=cst_t[:, _C_IC : _C_IC + BPC])
            expd = sml.tile([T, BPC], _F32, tag="expd")
            nc.scalar.activation(out=expd[:], in_=bp[:], func=mybir.ActivationFunctionType.Exp)
            emask = sml.tile([T, BPC], _F32, tag="emask")
            nc.vector.tensor_mul(out=emask[:], in0=expd[:], in1=cst_t[:, _C_KM : _C_KM + BPC])
            s_ps = ps2.tile([T, BPC], _F32)
            nc.tensor.matmul(s_ps[:], lhsT=cst_t[:, _C_UM : _C_UM + T], rhs=emask[:],
                             start=True, stop=True)
            s_sb = sml.tile([T, BPC], _F32, tag="s_sb")
            nc.vector.tensor_add(out=s_sb[:], in0=s_ps[:], in1=cst_t[:, _C_PD : _C_PD + BPC])
            logs = sml.tile([T, BPC], _F32, tag="logs")
            nc.scalar.activation(out=logs[:], in_=s_sb[:], func=mybir.ActivationFunctionType.Ln)
            t1 = sml.tile([T, BPC], _F32, tag="t1")
            nc.vector.tensor_mul(out=t1[:], in0=logs[:], in1=cst_t[:, _C_VM : _C_VM + BPC])
            t2 = sml.tile([T, BPC], _F32, tag="t2")
            nc.vector.tensor_mul(out=t2[:], in0=bp[:], in1=cst_t[:, _C_K1 : _C_K1 + BPC])
            diff = sml.tile([T, BPC], _F32, tag="diff")
            nc.vector.tensor_sub(out=diff[:], in0=t1[:], in1=t2[:])
            o_ps = ps2.tile([BPC, 1], _F32)
            nc.tensor.matmul(o_ps[:], lhsT=diff[:], rhs=cst_t[:, _C_ON : _C_ON + 1],
                             start=True, stop=True)
            o_sb = sml.tile([BPC, 1], _F32, tag="o_sb")
            nc.scalar.copy(out=o_sb[:], in_=o_ps[:])
            nc.scalar.dma_start(out=out[:], in_=o_sb[:])

    nc.compile()
    return nc


def _host_prep(inputs):
    enc = np.asarray(inputs["encoder_output"], dtype=np.float32)
    ends = np.asarray(inputs["his_turn_end_ids"]).astype(np.int64)
    lens = np.asarray(inputs["turn_lengths"]).astype(np.int64)
    w_fc = np.asarray(inputs["W_fc"], dtype=np.float32)
    w_b = w_fc[0, H:]  # [D]

    # per-sample needed chunks; sort desc, straight-deal to cores
    need = np.array(
        [int(np.ceil((ends[b, lens[b] - 1] + 1) / 128)) for b in range(B)], np.int64
    )
    order = np.argsort(-need, kind="stable")  # rank -> sample
    assign = order.reshape(BPC, N_CORES)  # [slot, core]
    slot_chunks = tuple(int(need[assign[k]].max()) for k in range(BPC))
    totc = sum(slot_chunks)

    # fp8 cast + chunk swizzle: [B, 128, NCHUNK, D]
    enc_q = enc.astype(ml_dtypes.float8_e4m3)
    x_sw = enc_q.reshape(B, NCHUNK, 128, D).transpose(0, 2, 1, 3)  # [B,128,NCHUNK,D]

    starts = np.concatenate([np.zeros((B, 1), np.int64), ends[:, :-1] + 1], axis=1)
    counts = (ends - starts + 1).astype(np.float32)
    s_idx = np.arange(S, dtype=np.int64)[None, :, None]
    mt_full = (
        (s_idx >= starts[:, None, :])
        & (s_idx <= ends[:, None, :])
        & (np.arange(T)[None, None, :] < lens[:, None, None])
    ).astype(ml_dtypes.float8_e4m3)  # exact 0/1 in fp8
    mt_sw = mt_full.reshape(B, NCHUNK, 128, T).transpose(0, 2, 3, 1)  # [B,128,T,NCHUNK]

    wbr = np.ascontiguousarray(np.broadcast_to(w_b[None, :], (128, D)))
    umat = (np.arange(T)[:, None] > np.arange(T)[None, :]).astype(np.float32)
    t_idx = np.arange(T)[:, None]  # [T, 1]
    inv_c = (1.0 / counts) * (t_idx.T < lens[:, None])  # [B, T]

    in_maps = []
    for ci in range(N_CORES):
        samples = [int(assign[k, ci]) for k in range(BPC)]
        xs = np.zeros((128, totc * CW), ml_dtypes.float8_e4m3)
        for b, g0, glen, col in _groups(slot_chunks):
            sb = samples[b]
            mt_blk = xs[:, col : col + glen * MTW].reshape(128, glen, MTW)
            x_blk = xs[:, col + glen * MTW : col + glen * CW].reshape(128, glen, D)
            for cc in range(glen):
                c = g0 + cc
                mt_blk[:, cc, :] = mt_sw[sb, :, :, c]
                x_blk[:, cc, :] = x_sw[sb, :, c, :]
        lc = lens[samples][None, :]  # [1, BPC]
        consts = np.empty((T, _C_W), np.float32)
        consts[:, _C_UM : _C_UM + T] = umat
        consts[:, _C_KM : _C_KM + BPC] = t_idx <= lc - 1
        consts[:, _C_K1 : _C_K1 + BPC] = (t_idx >= 1) & (t_idx <= lc - 1)
        consts[:, _C_VM : _C_VM + BPC] = t_idx <= lc - 2
        consts[:, _C_PD : _C_PD + BPC] = t_idx >= lc - 1
        consts[:, _C_IC : _C_IC + BPC] = inv_c[samples].T
        consts[:, _C_ON] = 1.0
        in_maps.append({"xm": xs, "wbr": wbr, "consts": consts})
    return in_maps, lens, slot_chunks


def kernel(**inputs) -> np.ndarray:
    global last_exec_time_ns, _nc_cache

    in_maps, lens, slot_chunks = _host_prep(inputs)

    if slot_chunks not in _nc_cache:
        _nc_cache[slot_chunks] = _build_nc(slot_chunks)
    nc = _nc_cache[slot_chunks]

    trace = bool(int(os.environ.get("KERNEL_TRACE", "0")))
    res = None
    last_err = None
    for _attempt in range(4):
        t = trace and _attempt == 0  # profiler can't restart after a fault
        try:
            res = run_bass_kernel_spmd(
                nc,
                in_maps,
                list(range(N_CORES)),
                trace=t,
                trace_cores=list(range(N_CORES)) if t else None,
            )
            break
        except Exception as e:  # transient first-run NRT faults; retry
            last_err = e
    if res is None:
        raise last_err
    last_exec_time_ns = res.exec_time_ns

    total = np.float64(0.0)
    for ci in range(N_CORES):
        total += np.sum(res.results[ci]["out"].astype(np.float64))
    denom = float(np.sum(lens - 1))
    return np.asarray(np.float32(total / denom))
